# revision 1
# baseline (speedup 1.0000x reference)
"""Trainium2 Bass kernel for quantized multi-head self-attention with PLA softmax.

Strategy (8 NeuronCores, data-parallel over batch, 2 batches/core):
  - Global fake-quant scales are host-computed (pure function of inputs) and
    shipped as a tiny fp32 parameter table (prm); the device program is
    straight-line and input-independent (compiled once, NEFF-cached).
  - All matmuls run on the PE in fp16 with integer-valued operands
    (quantized values are integers in [-127,127]); fp32 PSUM accumulation is
    then exact.  fp16 is used ONLY for integer-valued tensors; the softmax
    value chain is fp32 end-to-end (the triple int8 quantization of the
    reference makes the output extremely sensitive to sub-1e-4 relative
    errors in the PLA-softmax numerator/denominator).
  - The 12-segment PLA exp has the structure
        exps(t) = (dd + B_r) * M_r * h * e^{(i-12)h},   i=floor(t), r=i mod 3
    with M_r/B_r exact 3-point quadratics in (r-1).  On-device:
        iv = i-1 (fp32 magic floor, exact boundaries), J = floor(i/3) via an
        f16 round-to-int magic, rc = r-1, r2 = (r-1)^2 (all small ints, f16),
        za = iv + cp*J,  Et = Exp(sE*(za + g2*r2) + cE),
        ddb = t - kap*(za + gB*r2),  ex = (ddb + bpp)*Et  (accum -> den),
        pq = round(ex/den/sp) via the fp32 +-2^23*1.5 magic.
  - Only Identity/Exp/Abs-free activation funcs from ONE table are used, so
    there are no activation-table reloads (the baseline lost ~270us to
    Sin<->Exp table thrash).
"""

import sys
import numpy as np

sys.path.insert(0, "/opt/trn_rl_repo")

import concourse.bass as bass  # noqa: E402
import concourse.bacc as bacc  # noqa: E402
import concourse.mybir as mybir  # noqa: E402
from concourse import tile  # noqa: E402

try:
    import ml_dtypes

    BF16 = ml_dtypes.bfloat16
except ImportError:  # pragma: no cover
    BF16 = np.float16  # unused fallback

F16NP = np.float16

F32 = mybir.dt.float32
F16 = mybir.dt.float16

B, S, DM = 16, 512, 768
H, D = 12, 64
NCORES = 8
BLOC = B // NCORES          # 2 batches per core
T = BLOC * S                # 1024 tokens per core
SCALE = float(D) ** -0.5
QMAX = 127.0

NUM_INTERVALS = 12
DOMAIN_MIN, DOMAIN_MAX = -10.0, 0.0
PLA_H = (DOMAIN_MAX - DOMAIN_MIN) / NUM_INTERVALS  # 10/12

MAGIC = 12582912.0      # 1.5*2^23: RNE-to-integer magic
MAGH = 8388607.5        # 2^23 - 0.5 (fp32-exact)
MAGF1 = 8388609.0       # 2^23 + 1 (iv = m1 - MAGF1 = i - 1)
MINC = 8388619.0        # 2^23 + 11: clamp i <= 11
THIRD = float(np.float32(1.0 / 3.0))
C2MAG = float(np.float32(1023.6 + 1.0 / 3.0))


def _build_pla_coeffs():
    xs = np.linspace(DOMAIN_MIN, DOMAIN_MAX, 1001)
    ys = np.exp(xs)
    ivs = np.linspace(DOMAIN_MIN, DOMAIN_MAX, NUM_INTERVALS + 1)
    ms, cs = [], []
    for i in range(NUM_INTERVALS):
        mask = (xs >= ivs[i]) & (xs <= ivs[i + 1])
        m, c = np.polyfit(xs[mask], ys[mask], 1)
        ms.append(m)
        cs.append(c)
    return (
        np.asarray(ms, np.float32),
        np.asarray(cs, np.float32),
        np.asarray(ivs, np.float32),
    )


PLA_M, PLA_C, PLA_IVS = _build_pla_coeffs()

# exact 3-point (r-1)-centered quadratics of the per-residue wobble
_Mseg = (PLA_M / np.exp(PLA_IVS[:-1])).astype(np.float64)
_Bseg = ((PLA_M * PLA_IVS[:-1] + PLA_C) / (PLA_M * PLA_H)).astype(np.float64)
_lnM3 = np.log(_Mseg[:3])
_B3 = _Bseg[:3]
_bE = (_lnM3[2] - _lnM3[0]) / 2
_qE = (_lnM3[0] - 2 * _lnM3[1] + _lnM3[2]) / 2
_aE = _lnM3[1]
_bB = (_B3[2] - _B3[0]) / 2
_qB = (_B3[0] - 2 * _B3[1] + _B3[2]) / 2
_aB = _B3[1]
W_SE = float(PLA_H + _bE)
W_KAP = float(1.0 - _bB)
W_CP = float((-3 * _bE / W_SE + 3 * _bB / W_KAP) / 2)
W_G2 = float(_qE / W_SE)
W_GB = float(-_qB / W_KAP)
W_CE = float((-10.0 + np.log(PLA_H)) + _aE + PLA_H)
W_BPP = float(_aB - 1.0)


# ----------------------------------------------------------------------------
# Host-side reference replica (fp32 numpy): extracts global fake-quant scales.
# ----------------------------------------------------------------------------
def _qscale(x):
    return np.float32(max(np.float32(np.max(np.abs(x))) / np.float32(QMAX), 1e-8))


def _qint(x, s):
    return np.clip(np.round(np.asarray(x, np.float32) / s), -QMAX, QMAX).astype(
        np.float32
    )


def _host_scales(hidden, mask, Wq, bq, Wk, bk, Wv, bv, Wo, bo):
    h32 = np.asarray(hidden, np.float32)
    sh = _qscale(h32)
    xi = _qint(h32, sh)

    swq, swk, swv = _qscale(Wq), _qscale(Wk), _qscale(Wv)
    wqi, wki, wvi = _qint(Wq, swq), _qint(Wk, swk), _qint(Wv, swv)

    x2 = xi.reshape(-1, DM)
    q_lin = (x2 @ wqi.T) * np.float32(sh * swq) + np.asarray(bq, np.float32)
    k_lin = (x2 @ wki.T) * np.float32(sh * swk) + np.asarray(bk, np.float32)
    v_lin = (x2 @ wvi.T) * np.float32(sh * swv) + np.asarray(bv, np.float32)

    sq, sk, sv = _qscale(q_lin), _qscale(k_lin), _qscale(v_lin)
    qi = _qint(q_lin, sq).reshape(B, S, H, D).transpose(0, 2, 1, 3)
    ki = _qint(k_lin, sk).reshape(B, S, H, D).transpose(0, 2, 1, 3)
    vi = _qint(v_lin, sv).reshape(B, S, H, D).transpose(0, 2, 1, 3)

    lam = np.float32(np.float32(sq * sk) * np.float32(SCALE))
    si = np.matmul(qi, ki.transpose(0, 1, 3, 2))
    scores = si * lam + np.asarray(mask, np.float32)

    mx = np.max(scores, axis=-1, keepdims=True)
    d = (scores - mx).astype(np.float32)
    t = np.round(d * np.float32(2.0 ** 26))
    t = np.clip(t, -(2.0 ** 31), 2.0 ** 31 - 1).astype(np.float32) / np.float32(
        2.0 ** 26
    )
    xc = np.clip(t, np.float32(DOMAIN_MIN), np.float32(DOMAIN_MAX)).astype(np.float32)
    idx = np.clip(
        np.searchsorted(PLA_IVS, xc, side="right") - 1, 0, NUM_INTERVALS - 1
    )
    exps = PLA_M[idx] * xc + PLA_C[idx]
    den = np.sum(exps, axis=-1, keepdims=True) + np.float32(1e-9)
    probs = (exps / den).astype(np.float32)
    sp = _qscale(probs)
    pi = _qint(probs, sp)

    ctxi = np.matmul(pi, vi)
    ctx = (ctxi * np.float32(sp * sv)).transpose(0, 2, 1, 3).reshape(B, S, DM)
    sc = _qscale(ctx)
    swo = _qscale(Wo)

    return dict(
        sh=sh, swq=swq, swk=swk, swv=swv, swo=swo,
        sq=sq, sk=sk, sv=sv, sp=sp, sc=sc, xi=xi,
        wqi=wqi, wki=wki, wvi=wvi, lam=lam,
    )


# ----------------------------------------------------------------------------
# Device program (built once per process; input-independent)
# ----------------------------------------------------------------------------
_PROGRAMS = {}


def _build_program(use_mask):
    nc = bacc.Bacc(None, target_bir_lowering=False)

    xq_d = nc.dram_tensor("xq", [DM, T], F16, kind="ExternalInput")
    wq_d = nc.dram_tensor("wqT", [DM, DM], F16, kind="ExternalInput")
    wk_d = nc.dram_tensor("wkT", [DM, DM], F16, kind="ExternalInput")
    wv_d = nc.dram_tensor("wvT", [DM, DM], F16, kind="ExternalInput")
    wo_d = nc.dram_tensor("woT", [DM, DM], F16, kind="ExternalInput")
    maskdiv_d = nc.dram_tensor("maskdiv", [1, T], F16, kind="ExternalInput")
    bvdl_d = nc.dram_tensor("bvdl", [1, DM], F16, kind="ExternalInput")
    prm_d = nc.dram_tensor("prm", [128, 26], F32, kind="ExternalInput")
    ident_d = nc.dram_tensor("ident", [128, 128], F16, kind="ExternalInput")
    out_d = nc.dram_tensor("outT", [DM, T], F32, kind="ExternalOutput")

    AX = mybir.AxisListType.X
    OP = mybir.AluOpType
    AF = mybir.ActivationFunctionType

    with tile.TileContext(nc) as tc:
        with (
            tc.tile_pool(name="const", bufs=1) as cpool,
            tc.tile_pool(name="wts", bufs=1) as wpool,
            tc.tile_pool(name="acts", bufs=1) as apool,
            tc.tile_pool(name="work", bufs=2) as work,
            tc.tile_pool(name="stat", bufs=8) as stat,
            tc.tile_pool(name="psS", bufs=4, space="PSUM") as psS,
            tc.tile_pool(name="psT", bufs=2, space="PSUM") as psT,
            tc.tile_pool(name="psA", bufs=2, space="PSUM") as psA,
        ):
            # ---- constants / weights -----------------------------------
            prm0 = cpool.tile([128, 26], F32)
            nc.sync.dma_start(prm0[:], prm_d[:])
            prm = cpool.tile([128, 26], F32)
            nc.vector.tensor_copy(prm[:], prm0[:])
            bq2 = prm[:, 8:14]
            bk2 = prm[:, 14:20]
            bo2 = prm[:, 20:26]
            ident = cpool.tile([128, 128], F16)
            nc.sync.dma_start(ident[:], ident_d[:])
            maskdiv = cpool.tile([1, T], F16)
            nc.sync.dma_start(maskdiv[:], maskdiv_d[:])
            bvdl = cpool.tile([1, DM], F16)
            nc.sync.dma_start(bvdl[:], bvdl_d[:])
            ones1 = cpool.tile([1, 128], F16)
            nc.gpsimd.memset(ones1[:], 1.0)
            cNF1 = cpool.tile([128, 1], F32)
            nc.gpsimd.memset(cNF1[:], -MAGF1)
            cNMG = cpool.tile([128, 1], F32)
            nc.gpsimd.memset(cNMG[:], -MAGIC)
            cWCE = cpool.tile([128, 1], F32)
            nc.gpsimd.memset(cWCE[:], W_CE)

            xq = apool.tile([128, 6, T], F16, tag="xq")
            nc.sync.dma_start(xq[:], xq_d.rearrange("(c p) t -> p c t", p=128))
            wq = wpool.tile([128, 6, DM], F16)
            nc.sync.dma_start(wq[:], wq_d.rearrange("(c p) o -> p c o", p=128))
            wk = wpool.tile([128, 6, DM], F16)
            nc.sync.dma_start(wk[:], wk_d.rearrange("(c p) o -> p c o", p=128))
            wv = wpool.tile([128, 6, DM], F16)
            nc.sync.dma_start(wv[:], wv_d.rearrange("(c p) o -> p c o", p=128))
            wo = wpool.tile([128, 6, DM], F16)
            nc.sync.dma_start(wo[:], wo_d.rearrange("(c p) o -> p c o", p=128))

            # ---- Q/K projections (feature-major q^T, k^T) --------------
            qa = apool.tile([128, 6, T], F16, tag="qa")
            ka = apool.tile([128, 6, T], F16, tag="ka")

            def qk_tile(wt, bt2, rcol, dst, ot, th):
                ps = psA.tile([128, 512], F32, tag="mm")
                for kc in range(6):
                    nc.tensor.matmul(
                        ps[:],
                        wt[:, kc, ot * 128:(ot + 1) * 128],
                        xq[:, kc, th * 512:(th + 1) * 512],
                        start=(kc == 0),
                        stop=(kc == 5),
                    )
                raw = work.tile([128, 512], F32, tag="ev")
                nc.scalar.activation(
                    raw[:], ps[:], AF.Identity,
                    bias=bt2[:, ot:ot + 1],
                    scale=prm[:, rcol:rcol + 1],
                )
                nc.gpsimd.tensor_scalar(
                    dst[:, ot, th * 512:(th + 1) * 512], raw[:],
                    MAGIC, MAGIC, OP.add, OP.subtract,
                )

            def qk_proj_rest():
                for ot in range(6, 6):
                    for (wt, bt2, rcol, dst) in (
                        (wq, bq2, 0, qa), (wk, bk2, 1, ka)
                    ):
                        for th in range(2):
                            qk_tile(wt, bt2, rcol, dst, ot, th)
                            yield

            for ot in range(6):
                for (wt, bt2, rcol, dst) in ((wq, bq2, 0, qa), (wk, bk2, 1, ka)):
                    for th in range(2):
                        qk_tile(wt, bt2, rcol, dst, ot, th)

            # ---- V projection (token-major), emitted as a generator ----
            va = apool.tile([128, 8, DM], F16, tag="va")

            def v_proj_chain():
                for tt in range(8):
                    for oh in range(2):
                        ps = psA.tile([128, 512], F32, tag="mm")
                        for kc in range(6):
                            nc.tensor.matmul(
                                ps[:, 0:384],
                                xq[:, kc, tt * 128:(tt + 1) * 128],
                                wv[:, kc, oh * 384:(oh + 1) * 384],
                                start=(kc == 0),
                                stop=False,
                            )
                        nc.tensor.matmul(
                            ps[:, 0:384], ones1[0:1, :],
                            bvdl[0:1, oh * 384:(oh + 1) * 384],
                            start=False, stop=True,
                        )
                        raw = work.tile([128, 384], F32, tag="ev")
                        nc.scalar.activation(
                            raw[:], ps[:, 0:384], AF.Identity, bias=0.0,
                            scale=prm[:, 2:3],
                        )
                        nc.gpsimd.tensor_scalar(
                            va[:, tt, oh * 384:(oh + 1) * 384], raw[:],
                            MAGIC, MAGIC, OP.add, OP.subtract,
                        )
                        yield

            # ---- attention ---------------------------------------------
            # Two head-chains are emitted interleaved (generator round-robin)
            # so each engine's in-order queue alternates between independent
            # dependency chains instead of stalling on one.
            ca = wpool.tile([128, 8, DM], F16, tag="wk")  # ctx token-major

            def softmax_chain(b, hg, hh, ptq):
                h = 6 * hg + hh
                cq, off = h // 2, 64 * (h % 2)
                mx4 = stat.tile([128, 4], F32, tag="mx")
                bt4 = stat.tile([128, 4], F32, tag="bt")
                den4 = stat.tile([128, 4], F32, tag="den")
                sps = []
                for qt in range(4):
                    Sp = psS.tile([128, 512], F32, tag="sc")
                    nc.tensor.matmul(
                        Sp[:],
                        qa[off:off + 64, cq,
                           b * 512 + qt * 128: b * 512 + (qt + 1) * 128],
                        ka[off:off + 64, cq, b * 512:(b + 1) * 512],
                        start=True, stop=(not use_mask),
                    )
                    if use_mask:
                        nc.tensor.matmul(
                            Sp[:], ones1[0:1, :],
                            maskdiv[0:1, b * 512:(b + 1) * 512],
                            start=False, stop=True,
                        )
                    sps.append(Sp)
                    nc.vector.tensor_reduce(
                        mx4[:, qt:qt + 1], Sp[:], AX, OP.max
                    )
                    nc.vector.tensor_scalar(
                        bt4[:, qt:qt + 1], mx4[:, qt:qt + 1],
                        prm[:, 4:5], 12.0, OP.mult, OP.add,
                    )
                    yield
                tkb4 = work.tile([128, 4, 512], F32, tag="tk")
                for qt in range(4):
                    nc.scalar.activation(
                        tkb4[:, qt, :], sps[qt][:], AF.Identity,
                        bias=bt4[:, qt:qt + 1], scale=prm[:, 3:4],
                    )
                    if qt % 2:
                        yield
                exb4 = work.tile([128, 4, 512], F32, tag="ex")
                for p in range(2):
                    tkf = tkb4[:, 2 * p:2 * p + 2, :].rearrange(
                        "p a b -> p (a b)")
                    m1 = work.tile([128, 1024], F32, tag="m1")
                    nc.gpsimd.tensor_scalar(
                        m1[:], tkf, MAGH, MINC, OP.add, OP.min,
                    )
                    yield
                    ivb = work.tile([128, 1024], F16, tag="iv")
                    nc.gpsimd.tensor_scalar(
                        ivb[:], m1[:], MAGF1, None, OP.subtract,
                    )
                    yield
                    f2b = work.tile([128, 1024], F16, tag="fj")
                    nc.vector.tensor_scalar(
                        f2b[:], ivb[:], THIRD, C2MAG, OP.mult, OP.add
                    )
                    yield
                    f3m = work.tile([128, 1024], F16, tag="f3m")
                    nc.vector.tensor_scalar(
                        f3m[:], f2b[:], -3.0, -3072.0, OP.mult, OP.subtract
                    )
                    f3c = work.tile([128, 1024], F16, tag="f3")
                    nc.vector.tensor_scalar(
                        f3c[:], f2b[:], W_CP, 1024.0 * W_CP,
                        OP.mult, OP.subtract,
                    )
                    yield
                    rcb = work.tile([128, 1024], F16, tag="fj")
                    nc.vector.tensor_tensor(rcb[:], ivb[:], f3m[:], OP.add)
                    yield
                    r2b = work.tile([128, 1024], F16, tag="r2")
                    nc.vector.tensor_tensor(r2b[:], rcb[:], rcb[:], OP.mult)
                    yield
                    r2g = work.tile([128, 1024], F16, tag="fj")
                    nc.vector.tensor_scalar(
                        r2g[:], r2b[:], W_G2, None, OP.mult
                    )
                    r2d = work.tile([128, 1024], F16, tag="f3m")
                    nc.vector.tensor_scalar(
                        r2d[:], r2b[:], W_GB - W_G2, None, OP.mult
                    )
                    yield
                    w2b = work.tile([128, 1024], F16, tag="r2")
                    nc.vector.tensor_tensor(w2b[:], f3c[:], r2g[:], OP.add)
                    yield
                    za2 = work.tile([128, 1024], F32, tag="za2")
                    nc.gpsimd.tensor_tensor(za2[:], ivb[:], w2b[:], OP.add)
                    yield
                    zaB = work.tile([128, 1024], F32, tag="m1")
                    nc.gpsimd.tensor_tensor(zaB[:], za2[:], r2d[:], OP.add)
                    Et = work.tile([128, 2, 512], F32, tag="Et")
                    nc.scalar.activation(
                        Et.rearrange("p a b -> p (a b)"), za2[:],
                        AF.Exp, bias=cWCE[:, 0:1], scale=W_SE,
                    )
                    yield
                    ddb = work.tile([128, 2, 512], F32, tag="za2")
                    nc.vector.scalar_tensor_tensor(
                        ddb.rearrange("p a b -> p (a b)"), zaB[:], -W_KAP,
                        tkf, OP.mult, OP.add,
                    )
                    yield
                    for qp in range(2):
                        qt = 2 * p + qp
                        nc.vector.scalar_tensor_tensor(
                            exb4[:, qt, :], ddb[:, qp, :], W_BPP,
                            Et[:, qp, :], OP.add, OP.mult,
                            accum_out=den4[:, qt:qt + 1],
                        )
                    yield
                rc4 = stat.tile([128, 4], F32, tag="rc")
                nc.vector.reciprocal(rc4[:], den4[:])
                rcp4 = stat.tile([128, 4], F32, tag="rcp")
                nc.gpsimd.tensor_scalar(
                    rcp4[:], rc4[:], prm[:, 5:6], None, OP.mult
                )
                yield
                yb4 = work.tile([128, 4, 512], F32, tag="Et")
                for qt in range(4):
                    nc.vector.tensor_scalar(
                        yb4[:, qt, :], exb4[:, qt, :],
                        rcp4[:, qt:qt + 1], MAGIC, OP.mult, OP.add,
                    )
                    if qt % 2:
                        yield
                pq4 = work.tile([128, 4, 512], F16, tag="pq")
                for p in range(2):
                    nc.scalar.activation(
                        pq4[:, 2 * p:2 * p + 2, :].rearrange(
                            "p a b -> p (a b)"),
                        yb4[:, 2 * p:2 * p + 2, :].rearrange(
                            "p a b -> p (a b)"),
                        AF.Identity, bias=cNMG[:, 0:1], scale=1.0,
                    )
                    yield
                for kc in range(4):
                    pt_ps = psT.tile([128, 512], F16, tag="tps")
                    for qt in range(4):
                        nc.tensor.transpose(
                            pt_ps[:, qt * 128:(qt + 1) * 128],
                            pq4[:, qt, kc * 128:(kc + 1) * 128],
                            ident[:],
                        )
                    nc.scalar.copy(ptq[:, hh * 4 + kc, :], pt_ps[:])
                    yield

            for b in range(BLOC):
                for hg in range(2):          # head-groups of 6
                    ptq = wpool.tile([128, 24, 512], F16, tag="wq")
                    pending = [softmax_chain(b, hg, hh, ptq)
                               for hh in range(6)]
                    first = pending.pop(0)
                    alive = [first]
                    for _ in range(7):  # stagger: offset chain A by half
                        next(first)
                    alive.append(pending.pop(0))
                    if b == 0 and hg == 0:
                        def _proj_all():
                            yield from qk_proj_rest()
                            yield from v_proj_chain()
                        alive.append(_proj_all())
                    while alive:
                        nxt = []
                        for g in alive:
                            try:
                                next(g)
                                nxt.append(g)
                            except StopIteration:
                                if pending:
                                    g2 = pending.pop(0)
                                    try:
                                        next(g2)
                                        nxt.append(g2)
                                    except StopIteration:
                                        pass
                        alive = nxt
                    # ctx for this (b, head-group): writes cols 384*hg..+384
                    for qt in range(4):
                        cps = psA.tile([128, 512], F32, tag="mm")
                        for hh in range(6):
                            h = 6 * hg + hh
                            for kc in range(4):
                                nc.tensor.matmul(
                                    cps[:, hh * 64:(hh + 1) * 64],
                                    ptq[:, hh * 4 + kc, qt * 128:(qt + 1) * 128],
                                    va[:, b * 4 + kc, h * 64:(h + 1) * 64],
                                    start=(kc == 0), stop=(kc == 3),
                                )
                        craw = work.tile([128, 384], F32, tag="ev")
                        nc.scalar.activation(
                            craw[:], cps[:, 0:384], AF.Identity, bias=0.0,
                            scale=prm[:, 6:7],
                        )
                        nc.gpsimd.tensor_scalar(
                            ca[:, b * 4 + qt, hg * 384:(hg + 1) * 384],
                            craw[:], MAGIC, MAGIC, OP.add, OP.subtract,
                        )

            # ---- ctx transpose to feature-major ------------------------
            ctq = apool.tile([128, 6, T], F16, tag="xq")  # reuse xq slot
            for fc in range(6):
                for ttg in range(2):
                    pt2 = psT.tile([128, 512], F16, tag="tps")
                    for g in range(4):
                        tt = 4 * ttg + g
                        nc.tensor.transpose(
                            pt2[:, g * 128:(g + 1) * 128],
                            ca[:, tt, fc * 128:(fc + 1) * 128],
                            ident[:],
                        )
                    nc.vector.tensor_copy(
                        ctq[:, fc, ttg * 512:(ttg + 1) * 512], pt2[:]
                    )

            # ---- output projection (out^T feature-major) ---------------
            for ot in range(6):
                for th in range(2):
                    ps = psA.tile([128, 512], F32, tag="mm")
                    for fc in range(6):
                        nc.tensor.matmul(
                            ps[:],
                            wo[:, fc, ot * 128:(ot + 1) * 128],
                            ctq[:, fc, th * 512:(th + 1) * 512],
                            start=(fc == 0), stop=(fc == 5),
                        )
                    oev = work.tile([128, 512], F32, tag="ev")
                    nc.vector.tensor_scalar(
                        oev[:], ps[:], prm[:, 7:8], bo2[:, ot:ot + 1],
                        OP.mult, OP.add,
                    )
                    nc.sync.dma_start(
                        out_d.rearrange("(c p) t -> p c t", p=128)[
                            :, ot, th * 512:(th + 1) * 512
                        ],
                        oev[:],
                    )

    nc.compile()
    return nc


def _get_program(use_mask=False):
    key = bool(use_mask)
    if key not in _PROGRAMS:
        _PROGRAMS[key] = _build_program(key)
    return _PROGRAMS[key]


# ----------------------------------------------------------------------------
# Host <-> device marshalling
# ----------------------------------------------------------------------------
def make_in_maps(inputs, sc_):
    mask = np.asarray(inputs["attention_mask"], np.float32)
    bq = np.asarray(inputs["bq"], np.float32)
    bk = np.asarray(inputs["bk"], np.float32)
    bo = np.asarray(inputs["bo"], np.float32)
    Wo = np.asarray(inputs["Wo"], np.float32)

    woi = _qint(Wo, sc_["swo"])

    lamq = np.float32(sc_["sh"] * sc_["swq"])
    lamk = np.float32(sc_["sh"] * sc_["swk"])
    lamv = np.float32(sc_["sh"] * sc_["swv"])
    lam = sc_["lam"]

    prm = np.zeros((128, 26), np.float32)
    prm[:, 0] = lamq / sc_["sq"]
    prm[:, 1] = lamk / sc_["sk"]
    prm[:, 2] = lamv / sc_["sv"]
    prm[:, 3] = lam / np.float32(PLA_H)
    prm[:, 4] = -(lam / np.float32(PLA_H))
    prm[:, 5] = np.float32(1.0) / sc_["sp"]
    prm[:, 6] = np.float32(sc_["sp"] * sc_["sv"]) / sc_["sc"]
    prm[:, 7] = np.float32(sc_["sc"] * sc_["swo"])

    def _cols(vec, s):
        return np.ascontiguousarray(
            (vec.reshape(6, 128).T / np.float32(s)).astype(np.float32)
        )

    prm[:, 8:14] = _cols(bq, sc_["sq"])
    prm[:, 14:20] = _cols(bk, sc_["sk"])
    prm[:, 20:26] = bo.reshape(6, 128).T.astype(np.float32)
    bvdl = np.ascontiguousarray(
        (np.asarray(inputs["bv"], np.float32) / lamv).astype(F16NP).reshape(1, DM)
    )

    wqT = np.ascontiguousarray(sc_["wqi"].T.astype(F16NP))
    wkT = np.ascontiguousarray(sc_["wki"].T.astype(F16NP))
    wvT = np.ascontiguousarray(sc_["wvi"].T.astype(F16NP))
    woT = np.ascontiguousarray(woi.T.astype(F16NP))
    ident = np.eye(128, dtype=np.float32).astype(F16NP)

    in_maps = []
    for c in range(NCORES):
        xi_c = np.ascontiguousarray(
            sc_["xi"][2 * c:2 * c + 2].reshape(T, DM).T.astype(F16NP)
        )
        md_c = np.ascontiguousarray(
            (mask[2 * c:2 * c + 2, 0, 0, :] / lam).astype(F16NP).reshape(1, T)
        )
        in_maps.append({
            "xq": xi_c,
            "wqT": wqT, "wkT": wkT, "wvT": wvT, "woT": woT,
            "maskdiv": md_c, "bvdl": bvdl,
            "prm": prm, "ident": ident,
        })
    return in_maps


def assemble_output(per_core_outT):
    outs = []
    for c in range(NCORES):
        outT = np.asarray(per_core_outT[c], np.float32)
        outs.append(outT.T.reshape(BLOC, S, DM))
    out_lin = np.concatenate(outs, axis=0)
    so = _qscale(out_lin)
    q = np.clip(np.round(out_lin / so), -QMAX, QMAX) * so
    return q.astype(np.float32)


def kernel(**inputs) -> np.ndarray:
    sc_ = _host_scales(
        inputs["hidden_states"], inputs["attention_mask"],
        inputs["Wq"], inputs["bq"], inputs["Wk"], inputs["bk"],
        inputs["Wv"], inputs["bv"], inputs["Wo"], inputs["bo"],
    )
    in_maps = make_in_maps(inputs, sc_)
    use_mask = bool(np.any(np.asarray(inputs["attention_mask"], np.float32)))
    nc = _get_program(use_mask)

    from concourse.bass_utils import run_bass_kernel_spmd

    res = run_bass_kernel_spmd(nc, in_maps, list(range(NCORES)))
    return assemble_output([res.results[c]["outT"] for c in range(NCORES)])



# revision 40
# speedup vs baseline: 1.3643x; 1.3643x over previous
"""Trainium2 Bass kernel for quantized multi-head self-attention with PLA softmax.

Strategy (8 NeuronCores, data-parallel over batch, 2 batches/core):
  - Global fake-quant scales host-computed and shipped as a small fp32 table;
    device program is input-independent (compiled once).
  - All matmuls on PE in fp16 with integer-valued operands (exact in fp32 PSUM).
  - PLA softmax via the "two-magic" decomposition: the piecewise-linear exp
    table ex(t) = P_i*t + R_i (i = floor(t) in 12 segments) is represented as
        ex = (t/kap - zaB + bpp) * exp(sE*za2 + cE)
        za2 = iv + cpa*Ja + cpb*Jb,  zaB = za2 + dca*Ja + dcb*Jb
    where iv = floor(t)-1 (fp32 magic) and Ja/Jb are two phase-shifted f16
    round-to-int magics of iv/3.  The pair (Ja, Jb) spans the exact period-3
    least-squares wobble of the PLA coefficients, so the representation is
    exact up to a global scale (cancels in softmax normalization).
  - Probs are rounded to integers (fp32 magic) and transposed via the DMA
    crossbar (dma_start_transpose), freeing PE/ACT/DVE from transpose work.
  - Engine assignment balances DVE / Pool(GpSimd) / ACT; ~all ops are full
    [128, 2048]-per-chain passes.
"""

import sys
import numpy as np

sys.path.insert(0, "/opt/trn_rl_repo")

import concourse.bass as bass  # noqa: E402
import concourse.bacc as bacc  # noqa: E402
import concourse.mybir as mybir  # noqa: E402
from concourse import tile  # noqa: E402

F16NP = np.float16

F32 = mybir.dt.float32
F16 = mybir.dt.float16

B, S, DM = 16, 512, 768
H, D = 12, 64
NCORES = 8
BLOC = B // NCORES          # 2 batches per core
T = BLOC * S                # 1024 tokens per core
SCALE = float(D) ** -0.5
QMAX = 127.0

NUM_INTERVALS = 12
DOMAIN_MIN, DOMAIN_MAX = -10.0, 0.0
PLA_H = (DOMAIN_MAX - DOMAIN_MIN) / NUM_INTERVALS  # 10/12

MAGIC = 12582912.0      # 1.5*2^23: RNE-to-integer magic
MAGH = 8388607.5        # 2^23 - 0.5
MAGF1 = 8388609.0       # 2^23 + 1 (iv = m1 - MAGF1 = floor(t) - 1)
MINC = 8388619.0        # 2^23 + 11: clamp floor(t) <= 11
THIRD = float(np.float32(1.0 / 3.0))


def _build_pla_coeffs():
    xs = np.linspace(DOMAIN_MIN, DOMAIN_MAX, 1001)
    ys = np.exp(xs)
    ivs = np.linspace(DOMAIN_MIN, DOMAIN_MAX, NUM_INTERVALS + 1)
    ms, cs = [], []
    for i in range(NUM_INTERVALS):
        mask = (xs >= ivs[i]) & (xs <= ivs[i + 1])
        m, c = np.polyfit(xs[mask], ys[mask], 1)
        ms.append(m)
        cs.append(c)
    return (
        np.asarray(ms, np.float32),
        np.asarray(cs, np.float32),
        np.asarray(ivs, np.float32),
    )


PLA_M, PLA_C, PLA_IVS = _build_pla_coeffs()

# ---- two-magic decomposition constants (pure PLA-table solve, fp64) --------
_CA = float(np.float32(1024.6 + THIRD))          # f16 magic, ulp-1 zone
_CB = float(np.float32(1024.6))                  # phase-shifted by 1/3
_P64 = (PLA_M.astype(np.float64) * PLA_H)
_R64 = PLA_M.astype(np.float64) * (-10.0) + PLA_C.astype(np.float64)
_tau = -_R64 / _P64
_iv12 = (np.arange(12) - 1).astype(np.float64)


def _f16magic(ivv, C):
    return np.float16(
        np.float32(ivv) * np.float32(THIRD) + np.float32(C)
    ).astype(np.float64)


_Ja12 = _f16magic(_iv12, _CA)
_Jb12 = _f16magic(_iv12, _CB)
_A1 = np.stack([_iv12, _Ja12, _Jb12, np.ones(12)], axis=1)
_x1 = np.linalg.lstsq(_A1, np.log(_P64), rcond=None)[0]
W_SE = float(_x1[0])
W_CPA = float(_x1[1] / _x1[0])
W_CPB = float(_x1[2] / _x1[0])
_za12 = _iv12 + W_CPA * _Ja12 + W_CPB * _Jb12
_A2 = np.stack([_Ja12, _Jb12, np.ones(12), _tau], axis=1)
_x2 = np.linalg.lstsq(_A2, -_za12, rcond=None)[0]
W_DCA = float(_x2[0])
W_DCB = float(_x2[1])
W_BPP = float(-_x2[2])
W_KAP = float(1.0 / (-_x2[3]))
W_CE = float((np.log(W_KAP * _P64) - W_SE * _za12).mean())


# ----------------------------------------------------------------------------
# Host-side reference replica (fp32 numpy): extracts global fake-quant scales.
# ----------------------------------------------------------------------------
def _qscale(x):
    return np.float32(max(np.float32(np.max(np.abs(x))) / np.float32(QMAX), 1e-8))


def _qint(x, s):
    return np.clip(np.round(np.asarray(x, np.float32) / s), -QMAX, QMAX).astype(
        np.float32
    )


def _host_scales(hidden, mask, Wq, bq, Wk, bk, Wv, bv, Wo, bo):
    h32 = np.asarray(hidden, np.float32)
    sh = _qscale(h32)
    xi = _qint(h32, sh)

    swq, swk, swv = _qscale(Wq), _qscale(Wk), _qscale(Wv)
    wqi, wki, wvi = _qint(Wq, swq), _qint(Wk, swk), _qint(Wv, swv)

    x2 = xi.reshape(-1, DM)
    q_lin = (x2 @ wqi.T) * np.float32(sh * swq) + np.asarray(bq, np.float32)
    k_lin = (x2 @ wki.T) * np.float32(sh * swk) + np.asarray(bk, np.float32)
    v_lin = (x2 @ wvi.T) * np.float32(sh * swv) + np.asarray(bv, np.float32)

    sq, sk, sv = _qscale(q_lin), _qscale(k_lin), _qscale(v_lin)
    qi = _qint(q_lin, sq).reshape(B, S, H, D).transpose(0, 2, 1, 3)
    ki = _qint(k_lin, sk).reshape(B, S, H, D).transpose(0, 2, 1, 3)
    vi = _qint(v_lin, sv).reshape(B, S, H, D).transpose(0, 2, 1, 3)

    lam = np.float32(np.float32(sq * sk) * np.float32(SCALE))
    si = np.matmul(qi, ki.transpose(0, 1, 3, 2))
    scores = si * lam + np.asarray(mask, np.float32)

    mx = np.max(scores, axis=-1, keepdims=True)
    d = (scores - mx).astype(np.float32)
    t = np.round(d * np.float32(2.0 ** 26))
    t = np.clip(t, -(2.0 ** 31), 2.0 ** 31 - 1).astype(np.float32) / np.float32(
        2.0 ** 26
    )
    xc = np.clip(t, np.float32(DOMAIN_MIN), np.float32(DOMAIN_MAX)).astype(np.float32)
    idx = np.clip(
        np.searchsorted(PLA_IVS, xc, side="right") - 1, 0, NUM_INTERVALS - 1
    )
    exps = PLA_M[idx] * xc + PLA_C[idx]
    den = np.sum(exps, axis=-1, keepdims=True) + np.float32(1e-9)
    probs = (exps / den).astype(np.float32)
    sp = _qscale(probs)
    pi = _qint(probs, sp)

    ctxi = np.matmul(pi, vi)
    ctx = (ctxi * np.float32(sp * sv)).transpose(0, 2, 1, 3).reshape(B, S, DM)
    sc = _qscale(ctx)
    swo = _qscale(Wo)

    return dict(
        sh=sh, swq=swq, swk=swk, swv=swv, swo=swo,
        sq=sq, sk=sk, sv=sv, sp=sp, sc=sc, xi=xi,
        wqi=wqi, wki=wki, wvi=wvi, lam=lam,
    )


# ----------------------------------------------------------------------------
# Device program (built once per process; input-independent)
# ----------------------------------------------------------------------------
_PROGRAMS = {}


def _build_program(use_mask):
    nc = bacc.Bacc(None, target_bir_lowering=False)

    xq_d = nc.dram_tensor("xq", [DM, T], F16, kind="ExternalInput")
    wq_d = nc.dram_tensor("wqT", [DM, DM], F16, kind="ExternalInput")
    wk_d = nc.dram_tensor("wkT", [DM, DM], F16, kind="ExternalInput")
    wv_d = nc.dram_tensor("wvT", [DM, DM], F16, kind="ExternalInput")
    wo_d = nc.dram_tensor("woT", [DM, DM], F16, kind="ExternalInput")
    maskdiv_d = nc.dram_tensor("maskdiv", [1, T], F16, kind="ExternalInput")
    bvdl_d = nc.dram_tensor("bvdl", [1, DM], F16, kind="ExternalInput")
    prm_d = nc.dram_tensor("prm", [128, 26], F32, kind="ExternalInput")
    out_d = nc.dram_tensor("outT", [DM, T], F32, kind="ExternalOutput")

    AX = mybir.AxisListType.X
    OP = mybir.AluOpType
    AF = mybir.ActivationFunctionType

    with tile.TileContext(nc) as tc:
        with (
            tc.tile_pool(name="const", bufs=1) as cpool,
            tc.tile_pool(name="wts", bufs=1) as wpool,
            tc.tile_pool(name="acts", bufs=1) as apool,
            tc.tile_pool(name="p7", bufs=6) as p7,
            tc.tile_pool(name="r4", bufs=15) as r4,
            tc.tile_pool(name="r2", bufs=20) as r2,
            tc.tile_pool(name="stat", bufs=6) as stat,
            tc.tile_pool(name="psS", bufs=3, space="PSUM") as psS,
            tc.tile_pool(name="psA", bufs=2, space="PSUM") as psA,
        ):
            # ---- constants / weights -----------------------------------
            prm = cpool.tile([128, 26], F32)
            nc.sync.dma_start(prm[:], prm_d[:])
            bq2 = prm[:, 8:14]
            bk2 = prm[:, 14:20]
            bo2 = prm[:, 20:26]
            if use_mask:
                maskdiv = cpool.tile([1, T], F16)
                nc.sync.dma_start(maskdiv[:], maskdiv_d[:])
            bvdl = cpool.tile([1, DM], F16)
            nc.sync.dma_start(bvdl[:], bvdl_d[:])
            ones1 = cpool.tile([1, 128], F16)
            nc.gpsimd.memset(ones1[:], 1.0)
            cWCE = cpool.tile([128, 1], F32)
            nc.gpsimd.memset(cWCE[:], W_CE + W_SE * 1024.0 * (W_CPA + W_CPB))
            cNMG = cpool.tile([128, 1], F32)
            nc.gpsimd.memset(cNMG[:], -MAGIC)
            cPMG = cpool.tile([128, 1], F32)
            nc.gpsimd.memset(cPMG[:], MAGIC)
            cWA2 = cpool.tile([128, 1], F32)
            nc.gpsimd.memset(cWA2[:], -1024.0 * W_DCA)
            cWB2 = cpool.tile([128, 1], F32)
            nc.gpsimd.memset(cWB2[:], -1024.0 * W_DCB)

            wq = wpool.tile([128, 6, DM], F16)
            wk = wpool.tile([128, 6, DM], F16)
            wv = wpool.tile([128, 6, DM], F16)
            xq = apool.tile([128, 6, T], F16, tag="xq")
            xqr = xq_d.rearrange("(c p) t -> p c t", p=128)
            nc.sync.dma_start(wq[:], wq_d.rearrange("(c p) o -> p c o", p=128))
            nc.sync.dma_start(xq[:, :, 0:512], xqr[:, :, 0:512])
            nc.sync.dma_start(wk[:], wk_d.rearrange("(c p) o -> p c o", p=128))
            nc.sync.dma_start(xq[:, :, 512:1024], xqr[:, :, 512:1024])
            nc.sync.dma_start(wv[:], wv_d.rearrange("(c p) o -> p c o", p=128))

            # ---- Q/K projections (feature-major q^T, k^T) --------------
            qa = apool.tile([128, 6, T], F16, tag="qa")
            ka = apool.tile([128, 6, T], F16, tag="ka")

            def qk_tile(wt, bt2, rcol, dst, ot, th):
                ps = psA.tile([128, 512], F32, tag="mm")
                for kc in range(6):
                    nc.tensor.matmul(
                        ps[:],
                        wt[:, kc, ot * 128:(ot + 1) * 128],
                        xq[:, kc, th * 512:(th + 1) * 512],
                        start=(kc == 0),
                        stop=(kc == 5),
                    )
                raw = r4.tile([128, 512], F32, tag="r",
                               name=f"qkraw{ot}_{rcol}_{th}")
                nc.scalar.activation(
                    raw[:], ps[:], AF.Identity,
                    bias=bt2[:, ot:ot + 1],
                    scale=prm[:, rcol:rcol + 1],
                )
                nc.gpsimd.tensor_scalar(
                    dst[:, ot, th * 512:(th + 1) * 512], raw[:],
                    MAGIC, MAGIC, OP.add, OP.subtract,
                )

            def qk_chain():
                for th in range(2):
                    for ot in range(6):
                        for (wt, bt2, rcol, dst) in (
                            (wq, bq2, 0, qa), (wk, bk2, 1, ka)
                        ):
                            qk_tile(wt, bt2, rcol, dst, ot, th)
                            yield

            # ---- V projection (token-major), emitted as a generator ----
            va = apool.tile([128, 8, DM], F16, tag="va")

            def v_proj_chain():
                for tt in range(8):
                    for oh in range(2):
                        ps = psA.tile([128, 512], F32, tag="mm")
                        for kc in range(6):
                            nc.tensor.matmul(
                                ps[:, 0:384],
                                xq[:, kc, tt * 128:(tt + 1) * 128],
                                wv[:, kc, oh * 384:(oh + 1) * 384],
                                start=(kc == 0),
                                stop=False,
                            )
                        nc.tensor.matmul(
                            ps[:, 0:384], ones1[0:1, :],
                            bvdl[0:1, oh * 384:(oh + 1) * 384],
                            start=False, stop=True,
                        )
                        raw = r4.tile([128, 384], F32, tag="r",
                                       name=f"vraw{tt}_{oh}")
                        nc.scalar.activation(
                            raw[:], ps[:, 0:384], AF.Identity, bias=0.0,
                            scale=prm[:, 2:3],
                        )
                        nc.gpsimd.tensor_scalar(
                            va[:, tt, oh * 384:(oh + 1) * 384], raw[:],
                            MAGIC, MAGIC, OP.add, OP.subtract,
                        )
                        yield

            # ---- attention ---------------------------------------------
            ca = wpool.tile([128, 8, DM], F16, tag="wv")  # ctx token-major

            # ----------------------------------------------------------
            # Softmax pipeline: static stage schedule over 48 half-chain
            # units (b, hg, hh, half).  Each engine sees its per-unit ops
            # emitted in readiness order, so the in-order queues never
            # stall on a later unit's dependencies.
            # ----------------------------------------------------------
            ctq = apool.tile([128, 6, T], F16, tag="xq")  # reuse xq slot
            units = []
            hpi = 0
            ptqs = {}
            for b in range(BLOC):
                for hg in range(2):
                    for hp in range(3):
                        ptqs[(b, hg, hp)] = wpool.tile(
                            [128, 8, 512], F16, name=f"ptq{hpi}",
                            tag="wk" if (hpi % 2 or hpi >= 9) else "wq")
                        hpi += 1
                        for hh in (2 * hp, 2 * hp + 1):
                            for half in range(2):
                                units.append((b, hg, hh, half))

            NU = len(units)
            st = [dict() for _ in range(NU)]

            def s_scores(k):
                b, hg, hh, half = units[k]
                h = 6 * hg + hh
                cq, off = h // 2, 64 * (h % 2)
                Sp = psS.tile([128, 1024], F32, tag="sc")
                for qp in range(2):
                    qt = 2 * half + qp
                    nc.tensor.matmul(
                        Sp[:, qp * 512:(qp + 1) * 512],
                        qa[off:off + 64, cq,
                           b * 512 + qt * 128: b * 512 + (qt + 1) * 128],
                        ka[off:off + 64, cq, b * 512:(b + 1) * 512],
                        start=True, stop=(not use_mask),
                    )
                    if use_mask:
                        nc.tensor.matmul(
                            Sp[:, qp * 512:(qp + 1) * 512], ones1[0:1, :],
                            maskdiv[0:1, b * 512:(b + 1) * 512],
                            start=False, stop=True,
                        )
                st[k]["Sp"] = Sp

            def s_evict(k):
                Sp = st[k].pop("Sp")
                t0 = p7.tile([128, 2, 512], F32, tag="t0")
                nc.scalar.activation(
                    t0[:].rearrange("p a b -> p (a b)"),
                    Sp[:], AF.Identity, bias=0.0, scale=prm[:, 3:4],
                )
                st[k]["t0"] = t0

            def s_head(k):
                t0 = st[k]["t0"]
                mx2 = stat.tile([128, 2], F32, tag="mx")
                nc.vector.tensor_reduce(mx2[:], t0[:], AX, OP.max)
                bt2 = stat.tile([128, 2], F32, tag="bt")
                nc.vector.tensor_scalar(
                    bt2[:], mx2[:], -1.0, 12.0, OP.mult, OP.add)
                bpp2 = stat.tile([128, 2], F32, tag="bpp")
                nc.vector.tensor_scalar(
                    bpp2[:], bt2[:], 1.0 / W_KAP,
                    W_BPP - 1024.0 * (W_CPA + W_CPB + W_DCA + W_DCB),
                    OP.mult, OP.add)
                m1 = r4.tile([128, 2, 512], F32, tag="r", name=f"m1_{k}")
                for qp in range(2):
                    nc.vector.tensor_scalar(
                        m1[:, qp, :], t0[:, qp, :],
                        bt2[:, qp:qp + 1], MAGH, OP.add, OP.add)
                st[k].update(bt2=bt2, bpp2=bpp2, m1=m1)

            def s_iv(k):
                # Pool: iv = min(m1, MINC) - MAGF1 -> f16
                iv = r2.tile([128, 1024], F16, tag="r", name=f"iv_{k}")
                nc.gpsimd.tensor_scalar(
                    iv[:], st[k].pop("m1")[:].rearrange("p a b -> p (a b)"),
                    MINC, MAGF1, OP.min, OP.subtract)
                st[k]["iv"] = iv

            def s_mag(k):
                u = st[k]
                iv = u["iv"]
                Ja = r2.tile([128, 1024], F16, tag="r", name=f"Ja_{k}")
                nc.vector.tensor_scalar(
                    Ja[:], iv[:], THIRD, _CA, OP.mult, OP.add)
                Jb = r2.tile([128, 1024], F16, tag="r", name=f"Jb_{k}")
                nc.vector.tensor_scalar(
                    Jb[:], iv[:], THIRD, _CB, OP.mult, OP.add)
                # lambda-side small prescales (DVE f16)
                wa = r2.tile([128, 1024], F16, tag="r", name=f"wa_{k}")
                nc.vector.tensor_scalar(
                    wa[:], Ja[:], W_CPA, -1024.0 * W_CPA, OP.mult, OP.add)
                wb = r2.tile([128, 1024], F16, tag="r", name=f"wb_{k}")
                nc.vector.tensor_scalar(
                    wb[:], Jb[:], W_CPB, -1024.0 * W_CPB, OP.mult, OP.add)
                # tau-delta small prescales (ACT)
                wa2 = r2.tile([128, 1024], F16, tag="r", name=f"wa2_{k}")
                nc.scalar.activation(
                    wa2[:], Ja[:], AF.Identity,
                    bias=cWA2[:, 0:1], scale=W_DCA)
                wb2 = r2.tile([128, 1024], F16, tag="r", name=f"wb2_{k}")
                nc.scalar.activation(
                    wb2[:], Jb[:], AF.Identity,
                    bias=cWB2[:, 0:1], scale=W_DCB)
                u.update(Ja=Ja, Jb=Jb, wa=wa, wb=wb, wa2=wa2, wb2=wb2)

            def s_wz(k):
                u = st[k]
                w2 = r2.tile([128, 1024], F16, tag="r", name=f"w2_{k}")
                nc.gpsimd.tensor_tensor(
                    w2[:], u.pop("wa")[:], u.pop("wb")[:], OP.add)
                w4 = r2.tile([128, 1024], F16, tag="r", name=f"w4_{k}")
                nc.gpsimd.tensor_tensor(
                    w4[:], u.pop("wa2")[:], u.pop("wb2")[:], OP.add)
                u.pop("Ja"); u.pop("Jb")
                za2 = r4.tile([128, 1024], F32, tag="r", name=f"za2_{k}")
                nc.gpsimd.tensor_tensor(za2[:], u.pop("iv")[:], w2[:], OP.add)
                zaB = r4.tile([128, 1024], F32, tag="r", name=f"zaB_{k}")
                nc.gpsimd.tensor_tensor(zaB[:], za2[:], w4[:], OP.add)
                u.update(za2=za2, zaB=zaB)

            def s_Et(k):
                u = st[k]
                Et = r4.tile([128, 2, 512], F32, tag="r", name=f"Et_{k}")
                nc.scalar.activation(
                    Et[:].rearrange("p a b -> p (a b)"), u.pop("za2")[:],
                    AF.Exp, bias=cWCE[:, 0:1], scale=W_SE)
                # ddb = t0/kap - zaB  (DVE stt; bt/kap folded into ex scalar)
                t0 = u.pop("t0")
                ddb = r4.tile([128, 2, 512], F32, tag="r", name=f"ddb_{k}")
                nc.vector.scalar_tensor_tensor(
                    ddb[:].rearrange("p a b -> p (a b)"),
                    t0[:].rearrange("p a b -> p (a b)"),
                    1.0 / W_KAP, u.pop("zaB")[:], OP.mult, OP.subtract)
                u.update(Et=Et, ddb=ddb)

            def s_ex(k):
                u = st[k]
                den2 = stat.tile([128, 2], F32, tag="den")
                ex = r4.tile([128, 2, 512], F32, tag="r", name=f"ex_{k}")
                for qp in range(2):
                    nc.vector.scalar_tensor_tensor(
                        ex[:, qp, :], u["ddb"][:, qp, :],
                        u["bpp2"][:, qp:qp + 1],
                        u["Et"][:, qp, :], OP.add, OP.mult,
                        accum_out=den2[:, qp:qp + 1])
                rc2 = stat.tile([128, 2], F32, tag="rc")
                nc.vector.reciprocal(rc2[:], den2[:])
                rcp2 = stat.tile([128, 2], F32, tag="rcp")
                nc.vector.tensor_scalar(
                    rcp2[:], rc2[:], prm[:, 5:6], None, OP.mult)
                u.pop("ddb"); u.pop("Et"); u.pop("bpp2"); u.pop("bt2")
                st[k].update(ex=ex, rcp2=rcp2)

            def s_yb(k):
                u = st[k]
                yb = r4.tile([128, 2, 512], F32, tag="r", name=f"yb_{k}")
                for qp in range(2):
                    nc.gpsimd.tensor_scalar(
                        yb[:, qp, :], u["ex"][:, qp, :],
                        u["rcp2"][:, qp:qp + 1], MAGIC, OP.mult, OP.add)
                u.pop("ex"); u.pop("rcp2")
                st[k]["yb"] = yb

            def s_pq(k):
                u = st[k]
                pq2 = r2.tile([128, 2, 512], F16, tag="r", name=f"pq_{k}")
                nc.scalar.activation(
                    pq2[:].rearrange("p a b -> p (a b)"),
                    u.pop("yb")[:].rearrange("p a b -> p (a b)"),
                    AF.Identity, bias=cNMG[:, 0:1], scale=1.0)
                st[k]["pq2"] = pq2

            def s_tr(k):
                b, hg, hh, half = units[k]
                ptq = ptqs[(b, hg, hh // 2)]
                pq2 = st[k].pop("pq2")
                for kc in range(4):
                    for qp in range(2):
                        qt = 2 * half + qp
                        nc.sync.dma_start_transpose(
                            ptq[:, (hh % 2) * 4 + kc, qt * 128:(qt + 1) * 128],
                            pq2[:, qp, kc * 128:(kc + 1) * 128])

            def s_ctx(k):
                # runs when unit k is the LAST of its head-pair group
                b, hg, hh, half = units[k]
                if hh % 2 != 1 or half != 1:
                    return
                hp = hh // 2
                ptq = ptqs[(b, hg, hp)]
                cps = psA.tile([128, 4, 128], F32, tag="mm")
                for qt in range(4):
                    for hi in range(2):
                        h = 6 * hg + 2 * hp + hi
                        for kc in range(4):
                            nc.tensor.matmul(
                                cps[:, qt, hi * 64:(hi + 1) * 64],
                                ptq[:, hi * 4 + kc, qt * 128:(qt + 1) * 128],
                                va[:, b * 4 + kc, h * 64:(h + 1) * 64],
                                start=(kc == 0), stop=(kc == 3))
                st[k]["cps"] = cps

            def s_cev(k):
                b, hg, hh, half = units[k]
                if hh % 2 != 1 or half != 1:
                    return
                hp = hh // 2
                cps = st[k].pop("cps")
                craw = r4.tile([128, 4, 128], F32, tag="r",
                                name=f"craw_{k}")
                nc.scalar.activation(
                    craw[:].rearrange("p a b -> p (a b)"),
                    cps[:].rearrange("p a b -> p (a b)"),
                    AF.Identity, bias=0.0, scale=prm[:, 6:7])
                cb = (hg * 3 + hp) * 128
                nc.gpsimd.tensor_scalar(
                    ca[:, b * 4:(b + 1) * 4, cb:cb + 128],
                    craw[:], MAGIC, MAGIC, OP.add, OP.subtract)
                fc = hg * 3 + hp
                for tt in range(b * 4, b * 4 + 4):
                    nc.sync.dma_start_transpose(
                        ctq[:, fc, tt * 128:(tt + 1) * 128],
                        ca[:, tt, fc * 128:(fc + 1) * 128])

            # stage offsets: stage_fn(cycle - offset)
            # list order = within-cycle emission order (per-engine queue
            # order); offsets give the unit index per stage
            stages = [
                (0, s_scores), (1, s_evict), (4, s_mag), (2, s_head),
                (3, s_iv), (5, s_wz), (6, s_Et),
                (7, s_ex), (8, s_yb), (9, s_pq), (10, s_tr),
                (11, s_ctx), (13, s_cev),
            ]
            shared = {}

            def op_out(ot, th):
                wo = shared["wo"]
                ps = psA.tile([128, 512], F32, tag="mm")
                for fc in range(6):
                    nc.tensor.matmul(
                        ps[:],
                        wo[:, fc, ot * 128:(ot + 1) * 128],
                        ctq[:, fc, th * 512:(th + 1) * 512],
                        start=(fc == 0), stop=(fc == 5),
                    )
                oev = r4.tile([128, 512], F32, tag="r",
                               name=f"oev{ot}_{th}")
                nc.vector.tensor_scalar(
                    oev[:], ps[:], prm[:, 7:8], bo2[:, ot:ot + 1],
                    OP.mult, OP.add,
                )
                nc.sync.dma_start(
                    out_d.rearrange("(c p) t -> p c t", p=128)[
                        :, ot, th * 512:(th + 1) * 512
                    ],
                    oev[:],
                )

            vgen = v_proj_chain()
            qgen = qk_chain()
            LEAD = 2
            for c in range(NU + 15 + LEAD):
                for off, fn in stages:
                    k = c - off - LEAD
                    if 0 <= k < NU:
                        fn(k)
                for _ in range(2):
                    try:
                        next(qgen)
                    except StopIteration:
                        break
                try:
                    next(vgen)
                except StopIteration:
                    pass
                if c == 50:
                    wo = wpool.tile([128, 6, DM], F16, name="wo", tag="wq")
                    nc.sync.dma_start(
                        wo[:], wo_d.rearrange("(c p) o -> p c o", p=128))
                    shared["wo"] = wo
                if 53 <= c < 59:
                    op_out(c - 53, 0)

            # ---- output projection (woven into drain; see cycle loop) --
            for ot in range(6):
                op_out(ot, 1)

    nc.compile()
    return nc


def _get_program(use_mask=False):
    key = bool(use_mask)
    if key not in _PROGRAMS:
        _PROGRAMS[key] = _build_program(key)
    return _PROGRAMS[key]


# ----------------------------------------------------------------------------
# Host <-> device marshalling
# ----------------------------------------------------------------------------
def make_in_maps(inputs, sc_):
    mask = np.asarray(inputs["attention_mask"], np.float32)
    bq = np.asarray(inputs["bq"], np.float32)
    bk = np.asarray(inputs["bk"], np.float32)
    bo = np.asarray(inputs["bo"], np.float32)
    Wo = np.asarray(inputs["Wo"], np.float32)

    woi = _qint(Wo, sc_["swo"])

    lamq = np.float32(sc_["sh"] * sc_["swq"])
    lamk = np.float32(sc_["sh"] * sc_["swk"])
    lamv = np.float32(sc_["sh"] * sc_["swv"])
    lam = sc_["lam"]

    prm = np.zeros((128, 26), np.float32)
    prm[:, 0] = lamq / sc_["sq"]
    prm[:, 1] = lamk / sc_["sk"]
    prm[:, 2] = lamv / sc_["sv"]
    prm[:, 3] = lam / np.float32(PLA_H)
    prm[:, 4] = -(lam / np.float32(PLA_H))
    prm[:, 5] = np.float32(1.0) / sc_["sp"]
    prm[:, 6] = np.float32(sc_["sp"] * sc_["sv"]) / sc_["sc"]
    prm[:, 7] = np.float32(sc_["sc"] * sc_["swo"])

    def _cols(vec, s):
        return np.ascontiguousarray(
            (vec.reshape(6, 128).T / np.float32(s)).astype(np.float32)
        )

    prm[:, 8:14] = _cols(bq, sc_["sq"])
    prm[:, 14:20] = _cols(bk, sc_["sk"])
    prm[:, 20:26] = bo.reshape(6, 128).T.astype(np.float32)
    bvdl = np.ascontiguousarray(
        (np.asarray(inputs["bv"], np.float32) / lamv).astype(F16NP).reshape(1, DM)
    )

    wqT = np.ascontiguousarray(sc_["wqi"].T.astype(F16NP))
    wkT = np.ascontiguousarray(sc_["wki"].T.astype(F16NP))
    wvT = np.ascontiguousarray(sc_["wvi"].T.astype(F16NP))
    woT = np.ascontiguousarray(woi.T.astype(F16NP))

    in_maps = []
    for c in range(NCORES):
        xi_c = np.ascontiguousarray(
            sc_["xi"][2 * c:2 * c + 2].reshape(T, DM).T.astype(F16NP)
        )
        md_c = np.ascontiguousarray(
            (mask[2 * c:2 * c + 2, 0, 0, :] / lam).astype(F16NP).reshape(1, T)
        )
        in_maps.append({
            "xq": xi_c,
            "wqT": wqT, "wkT": wkT, "wvT": wvT, "woT": woT,
            "maskdiv": md_c, "bvdl": bvdl,
            "prm": prm,
        })
    return in_maps


def assemble_output(per_core_outT):
    outs = []
    for c in range(NCORES):
        outT = np.asarray(per_core_outT[c], np.float32)
        outs.append(outT.T.reshape(BLOC, S, DM))
    out_lin = np.concatenate(outs, axis=0)
    so = _qscale(out_lin)
    q = np.clip(np.round(out_lin / so), -QMAX, QMAX) * so
    return q.astype(np.float32)


def kernel(**inputs) -> np.ndarray:
    sc_ = _host_scales(
        inputs["hidden_states"], inputs["attention_mask"],
        inputs["Wq"], inputs["bq"], inputs["Wk"], inputs["bk"],
        inputs["Wv"], inputs["bv"], inputs["Wo"], inputs["bo"],
    )
    in_maps = make_in_maps(inputs, sc_)
    use_mask = bool(np.any(np.asarray(inputs["attention_mask"], np.float32)))
    nc = _get_program(use_mask)

    from concourse.bass_utils import run_bass_kernel_spmd

    res = run_bass_kernel_spmd(nc, in_maps, list(range(NCORES)))
    return assemble_output([res.results[c]["outT"] for c in range(NCORES)])
